# revision 1
# baseline (speedup 1.0000x reference)
"""Trainium2 Bass kernel for nn_DeRA_45389214384191.

Per-frame low-rank attention block:
  y = ( softmax( rope(q) rope(k)^T / sqrt(d) ) v  @ Wo.T + bo ) @ W_up.T
with q/k/v = (x @ W_down.T) @ W{q,k,v}.T + b{q,k,v}, attention strictly
per-frame (8 frames of 30*52=1560 tokens). One frame per NeuronCore.

All matmuls run in float32r (TF32-like, ~1.5e-4 rel err, full PE rate at
free-dim >= 256). RoPE is computed as q*C + swap(q)*S where swap(q) comes
from an extra projection with pair-swapped weight rows; the 1/sqrt(d)
softmax scale is folded into Wq/bq on the host, so one C/S table pair is
shared by q and k and by both padded head tiles. Softmax skips the
max-subtraction (scores are O(1) here; exp is exact fp32 on ACT) and gets
the denominator from a ones-column appended to V inside the PV matmul.
Compute-engine ops keep input/output partition bases aligned (no
cross-partition shifts); the only partition moves are DMAs.
"""

import numpy as np

_EXEC_CACHE = {}

# ---------------------------------------------------------------- config

DIM = 3072
RANK = 192
NH = 4
HD = 48          # head dim
HC = HD // 2     # complex pairs per head
T = 8            # frames
GH = 30
GW = 52
SEQ = T * GH * GW
FT = GH * GW     # tokens per frame = 1560
N_CORES = 8

# padded head layout for q/k: head h at rows PAD_OFF(h) of a 2x128 layout
PADR = 256


def _pad_off(h):
    return 128 * (h // 2) + 64 * (h % 2)


def _pad_map():
    m = np.zeros(RANK, dtype=np.int64)
    for r in range(RANK):
        h, j = divmod(r, HD)
        m[r] = _pad_off(h) + j
    return m


def _swap_perm():
    p = np.arange(RANK)
    return p.reshape(-1, 2)[:, ::-1].reshape(-1)


def _rope_tables(freqs_cos, freqs_sin, h, w):
    """Per-head C/S tables [HD, h*w]: q_rot = q * C + swap(q) * S."""
    s = h * w
    pc = np.zeros((HC, s), dtype=np.float64)
    ps = np.zeros((HC, s), dtype=np.float64)
    third = HC // 3
    hh = np.arange(s) // w
    ww = np.arange(s) % w
    pc[0:HC - 2 * third, :] = 1.0
    for j in range(HC - 2 * third, HC - third):
        pc[j, :] = freqs_cos[hh, j]
        ps[j, :] = freqs_sin[hh, j]
    for j in range(HC - third, HC):
        pc[j, :] = freqs_cos[ww, j]
        ps[j, :] = freqs_sin[ww, j]
    C = np.zeros((HD, s), dtype=np.float64)
    S = np.zeros((HD, s), dtype=np.float64)
    C[0::2, :] = pc
    C[1::2, :] = pc
    S[0::2, :] = -ps
    S[1::2, :] = ps
    return C, S


# ---------------------------------------------------------------- builder

def build_nc(ft=FT, dim=DIM, tn=390, un=384, phases=99, attn_bf16=False):
    import concourse.tile as tile
    from concourse import bacc, mybir

    fp32 = mybir.dt.float32

    ext = RANK + 1
    nc = bacc.Bacc(num_swdge_queues=4)
    dp = nc.declare_dram_parameter
    x_e = dp("xt", [dim, ft], fp32, isOutput=False)
    wd_e = dp("wd", [dim, RANK], fp32, isOutput=False)
    wq_e = dp("wq", [ext, PADR], fp32, isOutput=False)
    wk_e = dp("wk", [ext, PADR], fp32, isOutput=False)
    wqs_e = dp("wqs", [ext, PADR], fp32, isOutput=False)
    wks_e = dp("wks", [ext, PADR], fp32, isOutput=False)
    wv_e = dp("wv", [ext, RANK], fp32, isOutput=False)
    wo_e = dp("wo", [ext, RANK], fp32, isOutput=False)
    wu_e = dp("wu", [RANK, dim], fp32, isOutput=False)
    c_e = dp("ct", [128, ft], fp32, isOutput=False)
    s_e = dp("st", [128, ft], fp32, isOutput=False)
    y_e = dp("y", [ft, dim], fp32, isOutput=True)

    with tile.TileContext(nc) as tc:
        _build_body(nc, tc, mybir, ft, dim, tn, un,
                    x_e, wd_e, wq_e, wk_e, wqs_e, wks_e, wv_e, wo_e, wu_e,
                    c_e, s_e, y_e, phases, attn_bf16)
    nc.finalize()
    return nc


def _build_body(nc, tc, mybir, ft, dim, tn, un,
                x_e, wd_e, wq_e, wk_e, wqs_e, wks_e, wv_e, wo_e, wu_e,
                c_e, s_e, y_e, phases=99, attn_bf16=False):
    from contextlib import ExitStack

    fp32 = mybir.dt.float32
    fp32r = mybir.dt.float32r
    adt = mybir.dt.bfloat16 if attn_bf16 else fp32r
    AF = mybir.ActivationFunctionType

    kdim = dim // 128            # model-dim K-chunks
    ntch = ft // tn              # token chunks
    nkc = (ft + 127) // 128      # token K-chunks for attention / up-proj M
    nun = dim // un              # up-proj N-chunks
    ext = RANK + 1
    msz = (128, RANK - 128)      # rank M-tile sizes
    kcs = (128, ext - 128)       # xl K-chunk partition sizes

    root = ExitStack()
    with root:
        wpool = root.enter_context(tc.tile_pool(name="weights", bufs=1))
        csp = root.enter_context(tc.tile_pool(name="cs", bufs=1))
        xlp = root.enter_context(tc.tile_pool(name="xlp", bufs=1))
        qrp = root.enter_context(tc.tile_pool(name="qrp", bufs=1))
        vhep = root.enter_context(tc.tile_pool(name="vhep", bufs=1))
        oscp = root.enter_context(tc.tile_pool(name="oscp", bufs=1))
        o2p = root.enter_context(tc.tile_pool(name="o2p", bufs=1))
        es_a = ExitStack()
        wdp = es_a.enter_context(tc.tile_pool(name="wdpool", bufs=1))
        xp = es_a.enter_context(tc.tile_pool(name="xin", bufs=4))

        # ---------------- weight / table loads
        wd = []
        for k in range(kdim):
            t_ = wdp.tile([128, RANK], fp32r, tag=f"wd{k}", name=f"wd{k}")
            nc.gpsimd.dma_start(t_[:], wd_e[k * 128:(k + 1) * 128, :])
            wd.append(t_)

        def load_rows(e_, splits, cols, tag):
            out = []
            r0 = 0
            for i, rn in enumerate(splits):
                t_ = wpool.tile([rn, cols], fp32r, tag=f"{tag}{i}",
                                name=f"{tag}{i}")
                nc.gpsimd.dma_start(t_[:], e_[r0:r0 + rn, :])
                out.append(t_)
                r0 += rn
            return out

        # ---------------- phase A: down-projection -> xlT [ext, ft]
        xl = [xlp.tile([128, ft], fp32r, tag="xl0", name="xl0"),
              xlp.tile([ext - 128, ft], fp32r, tag="xl1", name="xl1")]
        nc.vector.memset(xl[1][ext - 129:ext - 128, :].bitcast(fp32), 1.0)

        with tc.tile_pool(name="psA", bufs=1, space="PSUM") as psA:
            ps = {}
            for mt in range(2):
                for nt in range(ntch):
                    ps[mt, nt] = psA.tile([msz[mt], tn], fp32,
                                          tag=f"a{mt}{nt}", name=f"a{mt}{nt}")
            for k in range(kdim):
                xt = xp.tile([128, ft], fp32r, tag="x", name="x")
                nc.gpsimd.dma_start(xt[:], x_e[k * 128:(k + 1) * 128, :])
                for mt in range(2):
                    for nt in range(ntch):
                        nc.tensor.matmul(
                            ps[mt, nt][:],
                            wd[k][:, mt * 128:mt * 128 + msz[mt]],
                            xt[:, nt * tn:(nt + 1) * tn],
                            start=(k == 0), stop=(k == kdim - 1))
            for mt in range(2):
                for nt in range(ntch):
                    nc.scalar.activation(
                        xl[mt][0:msz[mt], nt * tn:(nt + 1) * tn],
                        ps[mt, nt][:], AF.Copy)
        es_a.close()
        wq = load_rows(wq_e, kcs, PADR, "wq")
        wk = load_rows(wk_e, kcs, PADR, "wk")
        wqs = load_rows(wqs_e, kcs, PADR, "wqs")
        wks = load_rows(wks_e, kcs, PADR, "wks")
        wv = load_rows(wv_e, kcs, RANK, "wv")
        # out-proj K-chunks match the per-head o_scaled layout: 4x48 + ones
        wo = load_rows(wo_e, (HD,) * NH, RANK, "wo")
        bo_t = wpool.tile([128, 1], fp32, tag="bo", name="bo")
        nc.sync.dma_start(
            bo_t[:],
            wo_e[RANK:RANK + 1, 0:128].rearrange("a (p b) -> (a p) b", b=1))
        bo2_t = wpool.tile([RANK - 128, 1], fp32, tag="bo2", name="bo2")
        nc.sync.dma_start(
            bo2_t[:],
            wo_e[RANK:RANK + 1, 128:RANK].rearrange("a (p b) -> (a p) b", b=1))
        wu = load_rows(wu_e, (128, RANK - 128), dim, "wu")

        c_t = csp.tile([128, ft], fp32, tag="ct", name="ct")
        nc.sync.dma_start(c_t[:], c_e[:])
        s_t = csp.tile([128, ft], fp32, tag="st", name="st")
        nc.sync.dma_start(s_t[:], s_e[:])

        if phases < 2:
            return
        # ---------------- phase B: q/k/qswap/kswap (padded) + rope
        es_b = ExitStack()
        qkt = es_b.enter_context(tc.tile_pool(name="qkt", bufs=3))
        qr = [qrp.tile([128, ft], adt, tag="qr0", name="qr0"),
              qrp.tile([128, ft], adt, tag="qr1", name="qr1")]
        kr = [qrp.tile([128, ft], adt, tag="kr0", name="kr0"),
              qrp.tile([128, ft], adt, tag="kr1", name="kr1")]

        with tc.tile_pool(name="psB", bufs=1, space="PSUM") as psB:
            for (dst, wa, wb) in ((qr, wq, wqs), (kr, wk, wks)):
                for mt in range(2):
                    pb = [psB.tile([128, tn], fp32, tag=f"b{nt}",
                                   name=f"b{nt}") for nt in range(ntch)]
                    for k in range(2):
                        for nt in range(ntch):
                            nc.tensor.matmul(
                                pb[nt][:], wa[k][:, mt * 128:(mt + 1) * 128],
                                xl[k][:, nt * tn:(nt + 1) * tn],
                                start=(k == 0), stop=(k == 1))
                    pw = [psB.tile([128, tn], fp32, tag=f"w{nt}",
                                   name=f"w{nt}") for nt in range(ntch)]
                    for k in range(2):
                        for nt in range(ntch):
                            nc.tensor.matmul(
                                pw[nt][:], wb[k][:, mt * 128:(mt + 1) * 128],
                                xl[k][:, nt * tn:(nt + 1) * tn],
                                start=(k == 0), stop=(k == 1))
                    for nt in range(ntch):
                        nsl = slice(nt * tn, (nt + 1) * tn)
                        t1 = qkt.tile([128, tn], fp32, tag="t1", name="t1")
                        nc.vector.tensor_mul(t1[:], pb[nt][:], c_t[:, nsl])
                        t2 = qkt.tile([128, tn], fp32, tag="t2", name="t2")
                        nc.vector.tensor_mul(t2[:], pw[nt][:], s_t[:, nsl])
                        nc.vector.tensor_add(dst[mt][:, nsl], t1[:], t2[:])
        es_b.close()
        if phases < 3:
            return
        # ---------------- phase B2: v (token-major, head-grouped + ones col)
        vhe = []
        with tc.tile_pool(name="psV", bufs=2, space="PSUM") as psV:
            for kc in range(nkc):
                kn = min(128, ft - kc * 128)
                vt = vhep.tile([128, NH, HD + 1], adt, tag=f"vhe{kc}",
                               name=f"vhe{kc}")
                ps = psV.tile([128, RANK], fp32, tag="v", name="v")
                for k in range(2):
                    nc.tensor.matmul(
                        ps[0:kn, :], xl[k][:, kc * 128:kc * 128 + kn], wv[k][:],
                        start=(k == 0), stop=(k == 1))
                nc.scalar.activation(
                    vt[0:kn, :, 0:HD],
                    ps[0:kn, :].rearrange("p (n d) -> p n d", n=NH), AF.Copy)
                ones_ap = vt[0:kn, :, HD:HD + 1]
                if not attn_bf16:
                    ones_ap = ones_ap.bitcast(fp32)
                nc.vector.memset(ones_ap, 1.0)
                vhe.append(vt)

        if phases < 4:
            return
        # ---------------- phase C: attention (qc-pair outer for D/E overlap)
        es_c = ExitStack()
        pp = es_c.enter_context(tc.tile_pool(name="pexp", bufs=6))
        smallp = es_c.enter_context(tc.tile_pool(name="small", bufs=3))
        osc = [oscp.tile([HD, ft], fp32r, tag=f"osc{h}", name=f"osc{h}")
               for h in range(NH)]
        o2 = [o2p.tile([128, ft], fp32r, tag="o20", name="o20"),
              o2p.tile([RANK - 128, ft], fp32r, tag="o21", name="o21")]
        bo_tiles = (bo_t, bo2_t)
        nqp = ntch // 2
        with (
            tc.tile_pool(name="psS", bufs=2, space="PSUM") as psS,
            tc.tile_pool(name="psO", bufs=1, space="PSUM") as psO,
            tc.tile_pool(name="psD", bufs=1, space="PSUM") as psD,
        ):
            for qp in range(nqp):
                q0 = 2 * qp * tn
                for h in range(NH):
                    po = _pad_off(h)
                    ti, ro = po // 128, po % 128
                    opsum = [psO.tile([HD + 1, tn], fp32, tag=f"o{j}",
                                      name=f"o{j}") for j in range(2)]
                    for kc in range(nkc):
                        kn = min(128, ft - kc * 128)
                        st = psS.tile([128, 2, 512], fp32, tag="s", name="s")
                        for j in range(2):
                            nc.tensor.matmul(
                                st[0:kn, j, 0:tn],
                                kr[ti][ro:ro + HD, kc * 128:kc * 128 + kn],
                                qr[ti][ro:ro + HD, q0 + j * tn:q0 + (j + 1) * tn],
                                start=True, stop=True)
                        pt = pp.tile([128, 2, tn], fp32r, tag="p", name="p")
                        nc.scalar.activation(pt[0:kn, :, :],
                                             st[0:kn, :, 0:tn], AF.Exp)
                        for j in range(2):
                            nc.tensor.matmul(
                                opsum[j][:], vhe[kc][0:kn, h, :],
                                pt[0:kn, j, :],
                                start=(kc == 0), stop=(kc == nkc - 1))
                    lpair = smallp.tile([2, tn], fp32, tag="lp",
                                        name="lp", bufs=2)
                    lts = []
                    for j in range(2):
                        lt = smallp.tile([HD + 1, tn], fp32, tag="l", name="l",
                                         bufs=3)
                        nc.vector.tensor_copy(lt[:], opsum[j][:])
                        nc.gpsimd.dma_start(lpair[j:j + 1, :], lt[HD:HD + 1, :])
                        lts.append(lt)
                    rpair = smallp.tile([2, tn], fp32, tag="rp", name="rp",
                                        bufs=2)
                    nc.vector.reciprocal(rpair[:], lpair[:])
                    for j in range(2):
                        r0 = smallp.tile([1, tn], fp32, tag="r0", name="r0",
                                         bufs=2)
                        nc.gpsimd.dma_start(r0[:], rpair[j:j + 1, :])
                        bt = smallp.tile([HD, tn], fp32, tag="bl", name="bl",
                                         bufs=2)
                        nc.gpsimd.partition_broadcast(bt[:], r0[:])
                        nc.vector.tensor_mul(
                            osc[h][:, q0 + j * tn:q0 + (j + 1) * tn],
                            lts[j][0:HD, :], bt[:])
                # ---- phase D for this qc-pair
                for mt in range(2):
                    for j in range(2):
                        nsl = slice(q0 + j * tn, q0 + (j + 1) * tn)
                        ps = psD.tile([128, tn], fp32, tag="d", name="d")
                        for w in range(NH):
                            nc.tensor.matmul(
                                ps[0:msz[mt], :],
                                wo[w][:, mt * 128:mt * 128 + msz[mt]],
                                osc[w][:, nsl],
                                start=(w == 0), stop=(w == NH - 1))
                        nc.scalar.activation(
                            o2[mt][:, nsl], ps[0:msz[mt], :], AF.Identity,
                            bias=bo_tiles[mt][:])

        es_c.close()
        if phases < 6:
            return
        # ---------------- phase E: up-projection -> y [ft, dim]
        yp = root.enter_context(tc.tile_pool(name="yout", bufs=4))
        with tc.tile_pool(name="psU", bufs=1, space="PSUM") as psU:
            for mt in range(nkc):
                mn = min(128, ft - mt * 128)
                ps = [psU.tile([128, un], fp32, tag=f"u{ui}", name=f"u{ui}")
                      for ui in range(nun)]
                for k in range(2):
                    for ui in range(nun):
                        nc.tensor.matmul(
                            ps[ui][0:mn, :],
                            o2[k][:, mt * 128:mt * 128 + mn],
                            wu[k][:, ui * un:(ui + 1) * un],
                            start=(k == 0), stop=(k == 1))
                for ui in range(nun):
                    yt = yp.tile([128, un], fp32, tag="y", name="y")
                    if ui % 2 == 0:
                        nc.vector.tensor_copy(yt[0:mn, :], ps[ui][0:mn, :])
                    else:
                        nc.scalar.activation(yt[0:mn, :], ps[ui][0:mn, :],
                                             AF.Copy)
                    nc.sync.dma_start(
                        y_e[mt * 128:mt * 128 + mn, ui * un:(ui + 1) * un],
                        yt[0:mn, :])


# ---------------------------------------------------------------- host API

def _prep_inputs(x, freqs_cos, freqs_sin,
                 W_down, W_up, Wq, bq, Wk, bk, Wv, bv, Wo, bo,
                 ft=FT, n_cores=N_CORES, gh=GH, gw=GW):
    f32 = np.float32
    xT = np.ascontiguousarray(
        np.asarray(x, dtype=f32).reshape(-1, np.asarray(x).shape[-1]).T)

    pm = _pad_map()
    sw = _swap_perm()
    scale = HD ** -0.5

    def ext_w(W, b, pad, mul=1.0):
        We = np.concatenate(
            [np.asarray(W, np.float64).T, np.asarray(b, np.float64)[None, :]],
            axis=0) * mul
        if not pad:
            return np.ascontiguousarray(We.astype(f32))
        out = np.zeros((We.shape[0], PADR), dtype=np.float64)
        out[:, pm] = We
        return np.ascontiguousarray(out.astype(f32))

    C, S = _rope_tables(np.asarray(freqs_cos, np.float64),
                        np.asarray(freqs_sin, np.float64), gh, gw)

    def packed_cs(tab):
        out = np.zeros((128, ft), dtype=np.float64)
        out[0:HD, :] = tab
        out[64:64 + HD, :] = tab
        return np.ascontiguousarray(out.astype(f32))

    shared = dict(
        wd=np.ascontiguousarray(np.asarray(W_down, f32).T),
        wq=ext_w(Wq, bq, True, scale),
        wk=ext_w(Wk, bk, True),
        wqs=ext_w(np.asarray(Wq)[sw], np.asarray(bq)[sw], True, scale),
        wks=ext_w(np.asarray(Wk)[sw], np.asarray(bk)[sw], True),
        wv=ext_w(Wv, bv, False),
        wo=ext_w(Wo, bo, False),
        wu=np.ascontiguousarray(np.asarray(W_up, f32).T),
        ct=packed_cs(C), st=packed_cs(S),
    )
    in_maps = []
    for c in range(n_cores):
        m = dict(shared)
        m["xt"] = np.ascontiguousarray(xT[:, c * ft:(c + 1) * ft])
        in_maps.append(m)
    return in_maps


def kernel(x, seq_lens, t_size, h_size, w_size, sequence_cond_compressed_indices,
           freqs_cos, freqs_sin, W_down, W_up, Wq, bq, Wk, bk, Wv, bv, Wo, bo,
           _trace=False, _attn_bf16=False):
    from concourse.bass_utils import run_bass_kernel_spmd

    key = ("nc", _attn_bf16)
    if key not in _EXEC_CACHE:
        _EXEC_CACHE[key] = build_nc(attn_bf16=_attn_bf16)
    nc = _EXEC_CACHE[key]

    in_maps = _prep_inputs(x, freqs_cos, freqs_sin, W_down, W_up,
                           Wq, bq, Wk, bk, Wv, bv, Wo, bo)
    kwargs = {}
    if _trace:
        import concourse.bass_utils as bu
        bu.upload_artifacts = lambda tmpdir: tmpdir
        kwargs = dict(trace=True)
    res = run_bass_kernel_spmd(nc, in_maps, core_ids=list(range(N_CORES)), **kwargs)
    y = np.concatenate([res.results[c]["y"] for c in range(N_CORES)], axis=0)
    out = y[None, :, :].astype(np.float32)
    if _trace:
        return out, res
    return out



# revision 17
# speedup vs baseline: 1.2952x; 1.2952x over previous
"""Trainium2 Bass kernel for nn_DeRA_45389214384191.

Per-frame low-rank attention block, one frame (1560 tokens) per NeuronCore:
  y = ( softmax( rope(q) rope(k)^T / sqrt(d) ) v  @ Wo.T + bo ) @ W_up.T
with q/k/v = (x @ W_down.T) @ W{q,k,v}.T + b{q,k,v}.

v2 design:
- All matmul operands bf16 (host converts); PSUM accumulation fp32.
- Token chunks of TN=512 (PSUM-bank sized); ft = 512*3 + 24.
- Attention processed per head-pair: the two heads' score matmuls go to
  disjoint PE row groups (partition bases 0/64) and both land in one
  2-bank PSUM tile, so exp is a single ACT op per (pair, kc, jchunk).
- scores->exp->PV software-pipelined by one kc so PE never waits on ACT.
- Softmax denominator comes from a ones-column embedded in the V block at
  index 32 (so l lands on a 32-aligned PSUM partition). 1/l via the fast
  approx reciprocal, broadcast across the 49 block rows by gpsimd, one
  vector multiply normalizes; Wo carries a zero row at the slot so the
  junk row never reaches the output.
- Out-proj (D) and up-proj (E) run per jchunk, interleaved into the
  ACT-bound attention phase; y is written bf16.
"""

import numpy as np
import ml_dtypes

BF16 = ml_dtypes.bfloat16
_EXEC_CACHE = {}

# ---------------------------------------------------------------- config

DIM = 3072
RANK = 192
NH = 4
HD = 48          # head dim
HC = HD // 2     # complex pairs per head
T = 8            # frames
GH = 30
GW = 52
SEQ = T * GH * GW
FT = GH * GW     # tokens per frame = 1560
N_CORES = 8
PADR = 256       # padded q/k feature layout: head h at 128*(h//2)+64*(h%2)
EXT = RANK + 1   # xl rows incl. ones row
VB = HD + 1      # per-head V block width (ones col + 48 V cols)
OP = 0           # ones-column index inside the V block (l lands partition 0)
TN = 512         # token chunk


def _jchunks(ft=FT):
    return [(i * TN, min(TN, ft - i * TN)) for i in range((ft + TN - 1) // TN)]


def _kchunks(ft=FT):
    return [(i * 128, min(128, ft - i * 128)) for i in range((ft + 127) // 128)]


def _pad_off(h):
    return 128 * (h // 2) + 64 * (h % 2)


def _pad_map():
    m = np.zeros(RANK, dtype=np.int64)
    for r in range(RANK):
        h, j = divmod(r, HD)
        m[r] = _pad_off(h) + j
    return m


def _swap_perm():
    p = np.arange(RANK)
    return p.reshape(-1, 2)[:, ::-1].reshape(-1)


def _rope_tables(freqs_cos, freqs_sin, h, w):
    """Per-head C/S tables [HD, h*w]: q_rot = q * C + swap(q) * S."""
    s = h * w
    pc = np.zeros((HC, s), dtype=np.float64)
    ps = np.zeros((HC, s), dtype=np.float64)
    third = HC // 3
    hh = np.arange(s) // w
    ww = np.arange(s) % w
    pc[0:HC - 2 * third, :] = 1.0
    for j in range(HC - 2 * third, HC - third):
        pc[j, :] = freqs_cos[hh, j]
        ps[j, :] = freqs_sin[hh, j]
    for j in range(HC - third, HC):
        pc[j, :] = freqs_cos[ww, j]
        ps[j, :] = freqs_sin[ww, j]
    C = np.zeros((HD, s), dtype=np.float64)
    S = np.zeros((HD, s), dtype=np.float64)
    C[0::2, :] = pc
    C[1::2, :] = pc
    S[0::2, :] = -ps
    S[1::2, :] = ps
    return C, S


# ---------------------------------------------------------------- builder

def build_nc(ft=FT, dim=DIM):
    import concourse.tile as tile
    from concourse import bacc, mybir

    fp32 = mybir.dt.float32
    bf16 = mybir.dt.bfloat16

    nc = bacc.Bacc(num_swdge_queues=4)
    dp = nc.declare_dram_parameter
    x_e = dp("xt", [dim, ft], bf16, isOutput=False)
    wd_e = dp("wd", [dim, RANK], bf16, isOutput=False)
    wq_e = dp("wq", [EXT, PADR], bf16, isOutput=False)
    wk_e = dp("wk", [EXT, PADR], bf16, isOutput=False)
    wqs_e = dp("wqs", [EXT, PADR], bf16, isOutput=False)
    wks_e = dp("wks", [EXT, PADR], bf16, isOutput=False)
    wv_e = dp("wv", [EXT, NH * VB], bf16, isOutput=False)
    wo_e = dp("wo", [NH * VB, RANK], bf16, isOutput=False)
    bo_e = dp("bo", [1, RANK], fp32, isOutput=False)
    wu_e = dp("wu", [RANK, dim], bf16, isOutput=False)
    c_e = dp("ct", [128, ft], fp32, isOutput=False)
    s_e = dp("st", [128, ft], fp32, isOutput=False)
    y_e = dp("y", [ft, dim], bf16, isOutput=True)

    with tile.TileContext(nc) as tc:
        _build_body(nc, tc, mybir, ft, dim,
                    x_e, wd_e, wq_e, wk_e, wqs_e, wks_e, wv_e, wo_e, bo_e,
                    wu_e, c_e, s_e, y_e)
    nc.finalize()
    return nc


def _build_body(nc, tc, mybir, ft, dim,
                x_e, wd_e, wq_e, wk_e, wqs_e, wks_e, wv_e, wo_e, bo_e,
                wu_e, c_e, s_e, y_e):
    from contextlib import ExitStack

    fp32 = mybir.dt.float32
    bf16 = mybir.dt.bfloat16
    AF = mybir.ActivationFunctionType

    kdim = dim // 128
    JC = _jchunks(ft)
    KC = _kchunks(ft)
    nun = dim // TN              # up-proj N chunks (6)
    msz = (128, RANK - 128)      # rank M/K tile sizes (128, 64)
    kcs = (128, EXT - 128)       # xl K-chunk partition sizes (128, 65)

    root = ExitStack()
    with root:
        wpool = root.enter_context(tc.tile_pool(name="weights", bufs=1))
        csp = root.enter_context(tc.tile_pool(name="cs", bufs=1))
        xlp = root.enter_context(tc.tile_pool(name="xlp", bufs=1))
        qrp = root.enter_context(tc.tile_pool(name="qrp", bufs=1))
        vhep = root.enter_context(tc.tile_pool(name="vhep", bufs=1))
        oscp = root.enter_context(tc.tile_pool(name="oscp", bufs=1))
        o2p = root.enter_context(tc.tile_pool(name="o2p", bufs=1))
        es_a = ExitStack()
        wdp = es_a.enter_context(tc.tile_pool(name="wdpool", bufs=1))
        xp = es_a.enter_context(tc.tile_pool(name="xin", bufs=4))

        # ---------------- weight / table loads
        wd = []
        for k in range(kdim):
            t_ = wdp.tile([128, RANK], bf16, tag=f"wd{k}", name=f"wd{k}")
            nc.gpsimd.dma_start(t_[:], wd_e[k * 128:(k + 1) * 128, :])
            wd.append(t_)

        def load_rows(e_, splits, cols, tag):
            out = []
            r0 = 0
            for i, rn in enumerate(splits):
                t_ = wpool.tile([rn, cols], bf16, tag=f"{tag}{i}",
                                name=f"{tag}{i}")
                nc.sync.dma_start(t_[:], e_[r0:r0 + rn, :])
                out.append(t_)
                r0 += rn
            return out

        # ---------------- phase A: down-projection -> xlT [EXT, ft] bf16
        xl = [xlp.tile([128, ft], bf16, tag="xl0", name="xl0"),
              xlp.tile([EXT - 128, ft], bf16, tag="xl1", name="xl1")]
        nc.gpsimd.memset(xl[1][EXT - 129:EXT - 128, :], 1.0)

        with tc.tile_pool(name="psA", bufs=1, space="PSUM") as psA:
            ps = {}
            for mt in range(2):
                for nt in range(len(JC)):
                    ps[mt, nt] = psA.tile([msz[mt], TN], fp32,
                                          tag=f"a{mt}{nt}", name=f"a{mt}{nt}")
            for k in range(kdim):
                xt = xp.tile([128, ft], bf16, tag="x", name="x")
                eng = nc.gpsimd if k % 2 == 0 else nc.sync
                eng.dma_start(xt[:], x_e[k * 128:(k + 1) * 128, :])
                for mt in range(2):
                    for nt, (j0, jn) in enumerate(JC):
                        nc.tensor.matmul(
                            ps[mt, nt][0:msz[mt], 0:jn],
                            wd[k][:, mt * 128:mt * 128 + msz[mt]],
                            xt[:, j0:j0 + jn],
                            start=(k == 0), stop=(k == kdim - 1))
            for mt in range(2):
                for nt, (j0, jn) in enumerate(JC):
                    nc.scalar.activation(
                        xl[mt][0:msz[mt], j0:j0 + jn],
                        ps[mt, nt][0:msz[mt], 0:jn], AF.Copy)
        es_a.close()
        wq = load_rows(wq_e, kcs, PADR, "wq")
        wk = load_rows(wk_e, kcs, PADR, "wk")
        wqs = load_rows(wqs_e, kcs, PADR, "wqs")
        wks = load_rows(wks_e, kcs, PADR, "wks")
        wv = load_rows(wv_e, kcs, NH * VB, "wv")
        wo = load_rows(wo_e, (VB,) * NH, RANK, "wo")
        bo_t = wpool.tile([128, 1], fp32, tag="bo", name="bo")
        nc.sync.dma_start(
            bo_t[:],
            bo_e[0:1, 0:128].rearrange("a (p b) -> (a p) b", b=1))
        bo2_t = wpool.tile([RANK - 128, 1], fp32, tag="bo2", name="bo2")
        nc.sync.dma_start(
            bo2_t[:],
            bo_e[0:1, 128:RANK].rearrange("a (p b) -> (a p) b", b=1))
        wu = load_rows(wu_e, msz, dim, "wu")

        c_t = csp.tile([128, ft], fp32, tag="ct", name="ct")
        nc.sync.dma_start(c_t[:], c_e[:])
        s_t = csp.tile([128, ft], fp32, tag="st", name="st")
        nc.sync.dma_start(s_t[:], s_e[:])

        # ---------------- phase B: q/k/qswap/kswap (padded) + rope
        es_b = ExitStack()
        qkt = es_b.enter_context(tc.tile_pool(name="qkt", bufs=3))
        qr = [qrp.tile([128, ft], bf16, tag="qr0", name="qr0"),
              qrp.tile([128, ft], bf16, tag="qr1", name="qr1")]
        kr = [qrp.tile([128, ft], bf16, tag="kr0", name="kr0"),
              qrp.tile([128, ft], bf16, tag="kr1", name="kr1")]

        with tc.tile_pool(name="psB", bufs=1, space="PSUM") as psB:
            for (dst, wa, wb) in ((qr, wq, wqs), (kr, wk, wks)):
                for mt in range(2):
                    pb = [psB.tile([128, TN], fp32, tag=f"b{nt}",
                                   name=f"b{nt}") for nt in range(len(JC))]
                    for k in range(2):
                        for nt, (j0, jn) in enumerate(JC):
                            nc.tensor.matmul(
                                pb[nt][:, 0:jn],
                                wa[k][:, mt * 128:(mt + 1) * 128],
                                xl[k][:, j0:j0 + jn],
                                start=(k == 0), stop=(k == 1))
                    pw = [psB.tile([128, TN], fp32, tag=f"w{nt}",
                                   name=f"w{nt}") for nt in range(len(JC))]
                    for k in range(2):
                        for nt, (j0, jn) in enumerate(JC):
                            nc.tensor.matmul(
                                pw[nt][:, 0:jn],
                                wb[k][:, mt * 128:(mt + 1) * 128],
                                xl[k][:, j0:j0 + jn],
                                start=(k == 0), stop=(k == 1))
                    for nt, (j0, jn) in enumerate(JC):
                        nsl = slice(j0, j0 + jn)
                        t1 = qkt.tile([128, TN], fp32, tag="t1", name="t1")
                        nc.vector.tensor_mul(t1[:, 0:jn], pb[nt][:, 0:jn],
                                             c_t[:, nsl])
                        t2 = qkt.tile([128, TN], fp32, tag="t2", name="t2")
                        nc.vector.tensor_mul(t2[:, 0:jn], pw[nt][:, 0:jn],
                                             s_t[:, nsl])
                        nc.gpsimd.tensor_add(dst[mt][:, nsl], t1[:, 0:jn],
                                             t2[:, 0:jn])
        es_b.close()

        # ---------------- phase B2: v (token-major, blocked + ones col)
        vhe = []
        with tc.tile_pool(name="psV", bufs=2, space="PSUM") as psV:
            for kc, (k0, kn) in enumerate(KC):
                vt = vhep.tile([128, NH, VB], bf16, tag=f"vhe{kc}",
                               name=f"vhe{kc}")
                ps = psV.tile([128, NH * VB], fp32, tag="v", name="v")
                for k in range(2):
                    nc.tensor.matmul(
                        ps[0:kn, :], xl[k][:, k0:k0 + kn], wv[k][:],
                        start=(k == 0), stop=(k == 1))
                nc.scalar.activation(
                    vt[0:kn, :, :],
                    ps[0:kn, :].rearrange("p (n d) -> p n d", n=NH), AF.Copy)
                vhe.append(vt)

        # ---------------- phase C/D/E fused, per token chunk j
        # D and E for chunk jt are DEFERRED: their matmuls interleave into
        # chunk jt+1's scores/PV stream so the PE never idles during the
        # softmax-normalize handoff, and the tiny 24-token tail chunk's
        # stream gets filled with chunk 2's up-projection work.
        es_c = ExitStack()
        pp = es_c.enter_context(tc.tile_pool(name="pexp", bufs=3))
        ltp = es_c.enter_context(tc.tile_pool(name="ltp", bufs=2))
        rrp = es_c.enter_context(tc.tile_pool(name="rrp", bufs=2))
        rbp = es_c.enter_context(tc.tile_pool(name="rbp", bufs=2))
        yp = es_c.enter_context(tc.tile_pool(name="yout", bufs=4))
        o2 = [o2p.tile([128, ft], bf16, tag="o20", name="o20"),
              o2p.tile([RANK - 128, ft], bf16, tag="o21", name="o21")]
        bo_tiles = (bo_t, bo2_t)
        nkc = len(KC)
        with (
            tc.tile_pool(name="psS", bufs=2, space="PSUM") as psS,
            tc.tile_pool(name="psO", bufs=1, space="PSUM") as psO,
            tc.tile_pool(name="psM", bufs=2, space="PSUM") as psM,
        ):
            def de_tasks(j0, jn, osc_t):
                jsl = slice(j0, j0 + jn)
                tasks = []
                for mt in range(2):
                    def dtask(mt=mt):
                        pd = psM.tile([128, TN], fp32, tag="m", name="m")
                        for w in range(NH):
                            nc.tensor.matmul(
                                pd[0:msz[mt], 0:jn],
                                wo[w][:, mt * 128:mt * 128 + msz[mt]],
                                osc_t[w][0:VB, 0:jn],
                                start=(w == 0), stop=(w == NH - 1))
                        nc.scalar.activation(
                            o2[mt][:, jsl], pd[0:msz[mt], 0:jn], AF.Identity,
                            bias=bo_tiles[mt][:])
                    tasks.append(dtask)
                for mc in range(j0 // 128, (j0 + jn + 127) // 128):
                    for up in range(nun // 2):
                        def etask(mc=mc, up=up):
                            mn = min(128, ft - mc * 128)
                            pu = [psM.tile([128, TN], fp32, tag="m", name="m")
                                  for _ in range(2)]
                            for k in range(2):
                                for uu in range(2):
                                    ui = up * 2 + uu
                                    nc.tensor.matmul(
                                        pu[uu][0:mn, :],
                                        o2[k][:, mc * 128:mc * 128 + mn],
                                        wu[k][:, ui * TN:(ui + 1) * TN],
                                        start=(k == 0), stop=(k == 1))
                            for uu in range(2):
                                ui = up * 2 + uu
                                yt = yp.tile([128, TN], bf16, tag="y",
                                             name="y")
                                if uu == 0:
                                    nc.scalar.activation(
                                        yt[0:mn, :], pu[uu][0:mn, :], AF.Copy)
                                else:
                                    nc.vector.tensor_copy(yt[0:mn, :],
                                                          pu[uu][0:mn, :])
                                eng = nc.sync if uu == 0 else nc.gpsimd
                                eng.dma_start(
                                    y_e[mc * 128:mc * 128 + mn,
                                        ui * TN:(ui + 1) * TN],
                                    yt[0:mn, :])
                        tasks.append(etask)
                return tasks

            pending = []
            for jt, (j0, jn) in enumerate(JC):
                jsl = slice(j0, j0 + jn)
                tasks = pending
                pending = []
                osc_t = [oscp.tile([VB, TN], bf16, tag=f"osc{h}",
                                   name=f"osc{h}") for h in range(NH)]
                lts = {}
                for P in range(2):
                    o_ps = [psO.tile([VB, TN], fp32, tag=f"o{hh}",
                                     name=f"o{hh}") for hh in range(2)]
                    pts = {}

                    def emit_scores(kc):
                        k0, kn = KC[kc]
                        st = psS.tile([128, 2, TN], fp32, tag="s", name="s")
                        for hh in range(2):
                            base = 64 * hh
                            nc.tensor.matmul(
                                st[0:kn, hh, 0:jn],
                                kr[P][base:base + HD, k0:k0 + kn],
                                qr[P][base:base + HD, jsl],
                                start=True, stop=True)
                        pt = pp.tile([128, 2, TN], bf16, tag="p", name="p")
                        nc.scalar.activation(pt[0:kn, :, 0:jn],
                                             st[0:kn, :, 0:jn], AF.Exp)
                        pts[kc] = pt

                    def emit_pv(kc):
                        k0, kn = KC[kc]
                        for hh in range(2):
                            nc.tensor.matmul(
                                o_ps[hh][0:VB, 0:jn],
                                vhe[kc][0:kn, 2 * P + hh, :],
                                pts[kc][0:kn, hh, 0:jn],
                                start=(kc == 0), stop=(kc == nkc - 1))

                    for kc in range(nkc):
                        emit_scores(kc)
                        if kc > 0:
                            emit_pv(kc - 1)
                            if tasks:
                                tasks.pop(0)()
                    emit_pv(nkc - 1)
                    if tasks:
                        tasks.pop(0)()

                    # DVE-only part of normalize (no PE ops here so the next
                    # pair's scores don't stall behind it in the PE queue)
                    for hh in range(2):
                        h = 2 * P + hh
                        lt = ltp.tile([VB, TN], fp32, tag=f"lt{h}",
                                      name=f"lt{h}")
                        nc.vector.tensor_copy(lt[0:VB, 0:jn],
                                              o_ps[hh][0:VB, 0:jn])
                        rr = rrp.tile([1, TN], fp32, tag="rr", name="rr")
                        nc.vector.reciprocal_approx_fast(
                            rr[0:1, 0:jn], lt[OP:OP + 1, 0:jn])
                        lts[h] = (lt, rr)
                # broadcast 1/l across the block rows, normalize into osc
                for h in range(NH):
                    lt, rr = lts[h]
                    rb = rbp.tile([VB, TN], fp32, tag=f"rb{h % 2}", name="rb")
                    nc.gpsimd.partition_broadcast(rb[0:VB, 0:jn],
                                                  rr[0:1, 0:jn])
                    nc.vector.tensor_mul(osc_t[h][0:VB, 0:jn],
                                         lt[0:VB, 0:jn], rb[0:VB, 0:jn])
                for t in tasks:   # leftovers (shouldn't happen)
                    t()
                pending = de_tasks(j0, jn, osc_t)
            for t in pending:
                t()
        es_c.close()


# ---------------------------------------------------------------- host API

def _prep_inputs(x, freqs_cos, freqs_sin,
                 W_down, W_up, Wq, bq, Wk, bk, Wv, bv, Wo, bo,
                 ft=FT, n_cores=N_CORES, gh=GH, gw=GW, _qdtype=BF16):
    f64 = np.float64
    xT = np.ascontiguousarray(
        np.asarray(x, np.float32).reshape(-1, np.asarray(x).shape[-1]).T)

    pm = _pad_map()
    sw = _swap_perm()
    scale = HD ** -0.5

    def ext_w(W, b, mul=1.0):
        We = np.concatenate(
            [np.asarray(W, f64).T, np.asarray(b, f64)[None, :]],
            axis=0) * mul
        return We  # [EXT, RANK]

    def pad_cols(We):
        out = np.zeros((We.shape[0], PADR), dtype=f64)
        out[:, pm] = We
        return out

    C, S = _rope_tables(np.asarray(freqs_cos, f64),
                        np.asarray(freqs_sin, f64), gh, gw)

    def packed_cs(tab):
        out = np.zeros((128, ft), dtype=f64)
        out[0:HD, :] = tab
        out[64:64 + HD, :] = tab
        return np.ascontiguousarray(out.astype(np.float32))

    # V with per-head blocks [V0..31, ones, V32..47]
    Wve = ext_w(Wv, bv)
    wv_out = np.zeros((EXT, NH * VB), dtype=f64)
    for h in range(NH):
        blk = Wve[:, h * HD:(h + 1) * HD]
        wv_out[:, h * VB:h * VB + OP] = blk[:, 0:OP]
        wv_out[:, h * VB + OP + 1:h * VB + VB] = blk[:, OP:HD]
        wv_out[EXT - 1, h * VB + OP] = 1.0   # ones col via bias row
    # Wo with a zero row at the ones slot of each head block
    WoT = np.asarray(Wo, f64).T              # [RANK, RANK]
    wo_out = np.zeros((NH * VB, RANK), dtype=f64)
    for h in range(NH):
        blk = WoT[h * HD:(h + 1) * HD, :]
        wo_out[h * VB:h * VB + OP, :] = blk[0:OP, :]
        wo_out[h * VB + OP + 1:h * VB + VB, :] = blk[OP:HD, :]

    def b16(a):
        return np.ascontiguousarray(a.astype(_qdtype))

    shared = dict(
        wd=b16(np.asarray(W_down, f64).T),
        wq=b16(pad_cols(ext_w(Wq, bq, scale))),
        wk=b16(pad_cols(ext_w(Wk, bk))),
        wqs=b16(pad_cols(ext_w(np.asarray(Wq)[sw], np.asarray(bq)[sw],
                               scale))),
        wks=b16(pad_cols(ext_w(np.asarray(Wk)[sw], np.asarray(bk)[sw]))),
        wv=b16(wv_out),
        wo=b16(wo_out),
        bo=np.ascontiguousarray(np.asarray(bo, np.float32)[None, :]),
        wu=b16(np.asarray(W_up, f64).T),
        ct=packed_cs(C), st=packed_cs(S),
    )
    in_maps = []
    for c in range(n_cores):
        m = dict(shared)
        m["xt"] = b16(xT[:, c * ft:(c + 1) * ft].astype(f64))
        in_maps.append(m)
    return in_maps


def kernel(x, seq_lens, t_size, h_size, w_size, sequence_cond_compressed_indices,
           freqs_cos, freqs_sin, W_down, W_up, Wq, bq, Wk, bk, Wv, bv, Wo, bo,
           _trace=False):
    from concourse.bass_utils import run_bass_kernel_spmd

    key = "nc_v2"
    if key not in _EXEC_CACHE:
        _EXEC_CACHE[key] = build_nc()
    nc = _EXEC_CACHE[key]

    in_maps = _prep_inputs(x, freqs_cos, freqs_sin, W_down, W_up,
                           Wq, bq, Wk, bk, Wv, bv, Wo, bo)
    kwargs = {}
    if _trace:
        import concourse.bass_utils as bu
        bu.upload_artifacts = lambda tmpdir: tmpdir
        kwargs = dict(trace=True)
    res = run_bass_kernel_spmd(nc, in_maps, core_ids=list(range(N_CORES)), **kwargs)
    y = np.concatenate([np.asarray(res.results[c]["y"], dtype=np.float32)
                        for c in range(N_CORES)], axis=0)
    out = y[None, :, :]
    if _trace:
        return out, res
    return out


# revision 18
# speedup vs baseline: 1.5904x; 1.2279x over previous
"""Trainium2 Bass kernel for nn_DeRA_45389214384191.

Per-frame low-rank attention block, one frame (1560 tokens) per NeuronCore:
  y = ( softmax( rope(q) rope(k)^T / sqrt(d) ) v  @ Wo.T + bo ) @ W_up.T
with q/k/v = (x @ W_down.T) @ W{q,k,v}.T + b{q,k,v}.

v2 design:
- All matmul operands bf16 (host converts); PSUM accumulation fp32.
- Token chunks of TN=512 (PSUM-bank sized); ft = 512*3 + 24.
- Attention processed per head-pair: the two heads' score matmuls go to
  disjoint PE row groups (partition bases 0/64) and both land in one
  2-bank PSUM tile, so exp is a single ACT op per (pair, kc, jchunk).
- scores->exp->PV software-pipelined by one kc so PE never waits on ACT.
- Softmax denominator comes from a ones-column embedded in the V block at
  index 32 (so l lands on a 32-aligned PSUM partition). 1/l via the fast
  approx reciprocal, broadcast across the 49 block rows by gpsimd, one
  vector multiply normalizes; Wo carries a zero row at the slot so the
  junk row never reaches the output.
- Out-proj (D) and up-proj (E) run per jchunk, interleaved into the
  ACT-bound attention phase; y is written bf16.
"""

import numpy as np
import ml_dtypes

BF16 = ml_dtypes.bfloat16
_EXEC_CACHE = {}

# ---------------------------------------------------------------- config

DIM = 3072
RANK = 192
NH = 4
HD = 48          # head dim
HC = HD // 2     # complex pairs per head
T = 8            # frames
GH = 30
GW = 52
SEQ = T * GH * GW
FT = GH * GW     # tokens per frame = 1560
N_CORES = 8
PADR = 256       # padded q/k feature layout: head h at 128*(h//2)+64*(h%2)
EXT = RANK + 1   # xl rows incl. ones row
VB = HD + 1      # per-head V block width (ones col + 48 V cols)
OP = 0           # ones-column index inside the V block (l lands partition 0)
TN = 512         # token chunk


def _jchunks(ft=FT):
    return [(i * TN, min(TN, ft - i * TN)) for i in range((ft + TN - 1) // TN)]


def _kchunks(ft=FT):
    return [(i * 128, min(128, ft - i * 128)) for i in range((ft + 127) // 128)]


def _pad_off(h):
    return 128 * (h // 2) + 64 * (h % 2)


def _pad_map():
    m = np.zeros(RANK, dtype=np.int64)
    for r in range(RANK):
        h, j = divmod(r, HD)
        m[r] = _pad_off(h) + j
    return m


def _swap_perm():
    p = np.arange(RANK)
    return p.reshape(-1, 2)[:, ::-1].reshape(-1)


def _rope_tables(freqs_cos, freqs_sin, h, w):
    """Per-head C/S tables [HD, h*w]: q_rot = q * C + swap(q) * S."""
    s = h * w
    pc = np.zeros((HC, s), dtype=np.float64)
    ps = np.zeros((HC, s), dtype=np.float64)
    third = HC // 3
    hh = np.arange(s) // w
    ww = np.arange(s) % w
    pc[0:HC - 2 * third, :] = 1.0
    for j in range(HC - 2 * third, HC - third):
        pc[j, :] = freqs_cos[hh, j]
        ps[j, :] = freqs_sin[hh, j]
    for j in range(HC - third, HC):
        pc[j, :] = freqs_cos[ww, j]
        ps[j, :] = freqs_sin[ww, j]
    C = np.zeros((HD, s), dtype=np.float64)
    S = np.zeros((HD, s), dtype=np.float64)
    C[0::2, :] = pc
    C[1::2, :] = pc
    S[0::2, :] = -ps
    S[1::2, :] = ps
    return C, S


# ---------------------------------------------------------------- builder

def build_nc(ft=FT, dim=DIM):
    import concourse.tile as tile
    from concourse import bacc, mybir

    fp32 = mybir.dt.float32
    bf16 = mybir.dt.bfloat16

    nc = bacc.Bacc(num_swdge_queues=4)
    dp = nc.declare_dram_parameter
    x_e = dp("xt", [dim, ft], bf16, isOutput=False)
    wd_e = dp("wd", [dim, RANK], bf16, isOutput=False)
    wq_e = dp("wq", [EXT, PADR], bf16, isOutput=False)
    wk_e = dp("wk", [EXT, PADR], bf16, isOutput=False)
    wqs_e = dp("wqs", [EXT, PADR], bf16, isOutput=False)
    wks_e = dp("wks", [EXT, PADR], bf16, isOutput=False)
    wv_e = dp("wv", [EXT, NH * VB], bf16, isOutput=False)
    wo_e = dp("wo", [NH * VB, RANK], bf16, isOutput=False)
    bo_e = dp("bo", [1, RANK], fp32, isOutput=False)
    wu_e = dp("wu", [RANK, dim], bf16, isOutput=False)
    c_e = dp("ct", [128, ft], fp32, isOutput=False)
    s_e = dp("st", [128, ft], fp32, isOutput=False)
    y_e = dp("y", [ft, dim], bf16, isOutput=True)

    with tile.TileContext(nc) as tc:
        _build_body(nc, tc, mybir, ft, dim,
                    x_e, wd_e, wq_e, wk_e, wqs_e, wks_e, wv_e, wo_e, bo_e,
                    wu_e, c_e, s_e, y_e)
    nc.finalize()
    return nc


def _build_body(nc, tc, mybir, ft, dim,
                x_e, wd_e, wq_e, wk_e, wqs_e, wks_e, wv_e, wo_e, bo_e,
                wu_e, c_e, s_e, y_e):
    from contextlib import ExitStack

    fp32 = mybir.dt.float32
    bf16 = mybir.dt.bfloat16
    AF = mybir.ActivationFunctionType

    kdim = dim // 128
    JC = _jchunks(ft)
    KC = _kchunks(ft)
    nun = dim // TN              # up-proj N chunks (6)
    msz = (128, RANK - 128)      # rank M/K tile sizes (128, 64)
    kcs = (128, EXT - 128)       # xl K-chunk partition sizes (128, 65)

    root = ExitStack()
    with root:
        wpool = root.enter_context(tc.tile_pool(name="weights", bufs=1))
        csp = root.enter_context(tc.tile_pool(name="cs", bufs=1))
        xlp = root.enter_context(tc.tile_pool(name="xlp", bufs=1))
        qrp = root.enter_context(tc.tile_pool(name="qrp", bufs=1))
        vhep = root.enter_context(tc.tile_pool(name="vhep", bufs=1))
        oscp = root.enter_context(tc.tile_pool(name="oscp", bufs=1))
        o2p = root.enter_context(tc.tile_pool(name="o2p", bufs=1))
        es_a = ExitStack()
        wdp = es_a.enter_context(tc.tile_pool(name="wdpool", bufs=1))
        xp = es_a.enter_context(tc.tile_pool(name="xin", bufs=4))

        # ---------------- weight / table loads
        wd = []
        for k in range(kdim):
            t_ = wdp.tile([128, RANK], bf16, tag=f"wd{k}", name=f"wd{k}")
            nc.gpsimd.dma_start(t_[:], wd_e[k * 128:(k + 1) * 128, :])
            wd.append(t_)

        def load_rows(e_, splits, cols, tag):
            out = []
            r0 = 0
            for i, rn in enumerate(splits):
                t_ = wpool.tile([rn, cols], bf16, tag=f"{tag}{i}",
                                name=f"{tag}{i}")
                nc.sync.dma_start(t_[:], e_[r0:r0 + rn, :])
                out.append(t_)
                r0 += rn
            return out

        # ---------------- phase A: down-projection -> xlT [EXT, ft] bf16
        xl = [xlp.tile([128, ft], bf16, tag="xl0", name="xl0"),
              xlp.tile([EXT - 128, ft], bf16, tag="xl1", name="xl1")]
        nc.gpsimd.memset(xl[1][EXT - 129:EXT - 128, :], 1.0)

        with tc.tile_pool(name="psA", bufs=1, space="PSUM") as psA:
            ps = {}
            for mt in range(2):
                for nt in range(len(JC)):
                    ps[mt, nt] = psA.tile([msz[mt], TN], fp32,
                                          tag=f"a{mt}{nt}", name=f"a{mt}{nt}")
            for k in range(kdim):
                xt = xp.tile([128, ft], bf16, tag="x", name="x")
                eng = nc.gpsimd if k % 2 == 0 else nc.sync
                eng.dma_start(xt[:], x_e[k * 128:(k + 1) * 128, :])
                for mt in range(2):
                    for nt, (j0, jn) in enumerate(JC):
                        nc.tensor.matmul(
                            ps[mt, nt][0:msz[mt], 0:jn],
                            wd[k][:, mt * 128:mt * 128 + msz[mt]],
                            xt[:, j0:j0 + jn],
                            start=(k == 0), stop=(k == kdim - 1))
            for mt in range(2):
                for nt, (j0, jn) in enumerate(JC):
                    nc.scalar.activation(
                        xl[mt][0:msz[mt], j0:j0 + jn],
                        ps[mt, nt][0:msz[mt], 0:jn], AF.Copy)
        es_a.close()
        wq = load_rows(wq_e, kcs, PADR, "wq")
        wk = load_rows(wk_e, kcs, PADR, "wk")
        wqs = load_rows(wqs_e, kcs, PADR, "wqs")
        wks = load_rows(wks_e, kcs, PADR, "wks")
        wv = load_rows(wv_e, kcs, NH * VB, "wv")
        wo = load_rows(wo_e, (VB,) * NH, RANK, "wo")
        bo_t = wpool.tile([128, 1], fp32, tag="bo", name="bo")
        nc.sync.dma_start(
            bo_t[:],
            bo_e[0:1, 0:128].rearrange("a (p b) -> (a p) b", b=1))
        bo2_t = wpool.tile([RANK - 128, 1], fp32, tag="bo2", name="bo2")
        nc.sync.dma_start(
            bo2_t[:],
            bo_e[0:1, 128:RANK].rearrange("a (p b) -> (a p) b", b=1))
        wu = load_rows(wu_e, msz, dim, "wu")

        c_t = csp.tile([128, ft], fp32, tag="ct", name="ct")
        nc.sync.dma_start(c_t[:], c_e[:])
        s_t = csp.tile([128, ft], fp32, tag="st", name="st")
        nc.sync.dma_start(s_t[:], s_e[:])

        # ---------------- phase B: q/k/qswap/kswap (padded) + rope
        es_b = ExitStack()
        qkt = es_b.enter_context(tc.tile_pool(name="qkt", bufs=3))
        qr = [qrp.tile([128, ft], bf16, tag="qr0", name="qr0"),
              qrp.tile([128, ft], bf16, tag="qr1", name="qr1")]
        kr = [qrp.tile([128, ft], bf16, tag="kr0", name="kr0"),
              qrp.tile([128, ft], bf16, tag="kr1", name="kr1")]

        with tc.tile_pool(name="psB", bufs=1, space="PSUM") as psB:
            for (dst, wa, wb) in ((qr, wq, wqs), (kr, wk, wks)):
                for mt in range(2):
                    pb = [psB.tile([128, TN], fp32, tag=f"b{nt}",
                                   name=f"b{nt}") for nt in range(len(JC))]
                    for k in range(2):
                        for nt, (j0, jn) in enumerate(JC):
                            nc.tensor.matmul(
                                pb[nt][:, 0:jn],
                                wa[k][:, mt * 128:(mt + 1) * 128],
                                xl[k][:, j0:j0 + jn],
                                start=(k == 0), stop=(k == 1))
                    pw = [psB.tile([128, TN], fp32, tag=f"w{nt}",
                                   name=f"w{nt}") for nt in range(len(JC))]
                    for k in range(2):
                        for nt, (j0, jn) in enumerate(JC):
                            nc.tensor.matmul(
                                pw[nt][:, 0:jn],
                                wb[k][:, mt * 128:(mt + 1) * 128],
                                xl[k][:, j0:j0 + jn],
                                start=(k == 0), stop=(k == 1))
                    for nt, (j0, jn) in enumerate(JC):
                        nsl = slice(j0, j0 + jn)
                        t1 = qkt.tile([128, TN], fp32, tag="t1", name="t1")
                        nc.vector.tensor_mul(t1[:, 0:jn], pb[nt][:, 0:jn],
                                             c_t[:, nsl])
                        t2 = qkt.tile([128, TN], fp32, tag="t2", name="t2")
                        nc.vector.tensor_mul(t2[:, 0:jn], pw[nt][:, 0:jn],
                                             s_t[:, nsl])
                        nc.gpsimd.tensor_add(dst[mt][:, nsl], t1[:, 0:jn],
                                             t2[:, 0:jn])
        es_b.close()

        # ---------------- phase B2: v (token-major, blocked + ones col)
        vhe = []
        with tc.tile_pool(name="psV", bufs=2, space="PSUM") as psV:
            for kc, (k0, kn) in enumerate(KC):
                vt = vhep.tile([128, NH, VB], bf16, tag=f"vhe{kc}",
                               name=f"vhe{kc}")
                ps = psV.tile([128, NH * VB], fp32, tag="v", name="v")
                for k in range(2):
                    nc.tensor.matmul(
                        ps[0:kn, :], xl[k][:, k0:k0 + kn], wv[k][:],
                        start=(k == 0), stop=(k == 1))
                nc.scalar.activation(
                    vt[0:kn, :, :],
                    ps[0:kn, :].rearrange("p (n d) -> p n d", n=NH), AF.Copy)
                vhe.append(vt)

        # ---------------- phase C/D/E fused, per token chunk j
        # D and E for chunk jt are DEFERRED: their matmuls interleave into
        # chunk jt+1's scores/PV stream so the PE never idles during the
        # softmax-normalize handoff, and the tiny 24-token tail chunk's
        # stream gets filled with chunk 2's up-projection work.
        es_c = ExitStack()
        pp = es_c.enter_context(tc.tile_pool(name="pexp", bufs=3))
        ltp = es_c.enter_context(tc.tile_pool(name="ltp", bufs=2))
        rrp = es_c.enter_context(tc.tile_pool(name="rrp", bufs=2))
        rbp = es_c.enter_context(tc.tile_pool(name="rbp", bufs=2))
        yp = es_c.enter_context(tc.tile_pool(name="yout", bufs=4))
        o2 = [o2p.tile([128, ft], bf16, tag="o20", name="o20"),
              o2p.tile([RANK - 128, ft], bf16, tag="o21", name="o21")]
        bo_tiles = (bo_t, bo2_t)
        nkc = len(KC)
        with (
            tc.tile_pool(name="psS", bufs=2, space="PSUM") as psS,
            tc.tile_pool(name="psO", bufs=1, space="PSUM") as psO,
            tc.tile_pool(name="psM", bufs=2, space="PSUM") as psM,
        ):
            def de_tasks(j0, jn, osc_t):
                jsl = slice(j0, j0 + jn)
                tasks = []
                for mt in range(2):
                    def dtask(mt=mt):
                        pd = psM.tile([128, TN], fp32, tag="m", name="m")
                        for w in range(NH):
                            nc.tensor.matmul(
                                pd[0:msz[mt], 0:jn],
                                wo[w][:, mt * 128:mt * 128 + msz[mt]],
                                osc_t[w][0:VB, 0:jn],
                                start=(w == 0), stop=(w == NH - 1))
                        nc.scalar.activation(
                            o2[mt][:, jsl], pd[0:msz[mt], 0:jn], AF.Identity,
                            bias=bo_tiles[mt][:])
                    tasks.append(dtask)
                for mc in range(j0 // 128, (j0 + jn + 127) // 128):
                    for up in range(nun // 2):
                        def etask(mc=mc, up=up):
                            mn = min(128, ft - mc * 128)
                            pu = [psM.tile([128, TN], fp32, tag="m", name="m")
                                  for _ in range(2)]
                            for k in range(2):
                                for uu in range(2):
                                    ui = up * 2 + uu
                                    nc.tensor.matmul(
                                        pu[uu][0:mn, :],
                                        o2[k][:, mc * 128:mc * 128 + mn],
                                        wu[k][:, ui * TN:(ui + 1) * TN],
                                        start=(k == 0), stop=(k == 1))
                            for uu in range(2):
                                ui = up * 2 + uu
                                yt = yp.tile([128, TN], bf16, tag="y",
                                             name="y")
                                nc.vector.tensor_copy(yt[0:mn, :],
                                                      pu[uu][0:mn, :])
                                eng = nc.sync if uu == 0 else nc.gpsimd
                                eng.dma_start(
                                    y_e[mc * 128:mc * 128 + mn,
                                        ui * TN:(ui + 1) * TN],
                                    yt[0:mn, :])
                        tasks.append(etask)
                return tasks

            pending = []
            for jt, (j0, jn) in enumerate(JC):
                jsl = slice(j0, j0 + jn)
                tasks = pending
                pending = []
                osc_t = [oscp.tile([VB, TN], bf16, tag=f"osc{h}",
                                   name=f"osc{h}") for h in range(NH)]
                lts = {}
                for P in range(2):
                    o_ps = [psO.tile([VB, TN], fp32, tag=f"o{hh}",
                                     name=f"o{hh}") for hh in range(2)]
                    pts = {}

                    def emit_scores(kc):
                        k0, kn = KC[kc]
                        st = psS.tile([128, 2, TN], fp32, tag="s", name="s")
                        for hh in range(2):
                            base = 64 * hh
                            nc.tensor.matmul(
                                st[0:kn, hh, 0:jn],
                                kr[P][base:base + HD, k0:k0 + kn],
                                qr[P][base:base + HD, jsl],
                                start=True, stop=True)
                        pt = pp.tile([128, 2, TN], bf16, tag="p", name="p")
                        nc.scalar.activation(pt[0:kn, :, 0:jn],
                                             st[0:kn, :, 0:jn], AF.Exp)
                        pts[kc] = pt

                    def emit_pv(kc):
                        k0, kn = KC[kc]
                        for hh in range(2):
                            nc.tensor.matmul(
                                o_ps[hh][0:VB, 0:jn],
                                vhe[kc][0:kn, 2 * P + hh, :],
                                pts[kc][0:kn, hh, 0:jn],
                                start=(kc == 0), stop=(kc == nkc - 1))

                    for kc in range(nkc):
                        emit_scores(kc)
                        if kc > 0:
                            emit_pv(kc - 1)
                            if tasks:
                                tasks.pop(0)()
                    emit_pv(nkc - 1)
                    if tasks:
                        tasks.pop(0)()

                    # DVE-only part of normalize (no PE ops here so the next
                    # pair's scores don't stall behind it in the PE queue)
                    for hh in range(2):
                        h = 2 * P + hh
                        lt = ltp.tile([VB, TN], fp32, tag=f"lt{h}",
                                      name=f"lt{h}")
                        nc.vector.tensor_copy(lt[0:VB, 0:jn],
                                              o_ps[hh][0:VB, 0:jn])
                        rr = rrp.tile([1, TN], fp32, tag="rr", name="rr")
                        nc.vector.reciprocal_approx_fast(
                            rr[0:1, 0:jn], lt[OP:OP + 1, 0:jn])
                        lts[h] = (lt, rr)
                # broadcast 1/l across the block rows, normalize into osc
                for h in range(NH):
                    lt, rr = lts[h]
                    rb = rbp.tile([VB, TN], fp32, tag=f"rb{h % 2}", name="rb")
                    nc.gpsimd.partition_broadcast(rb[0:VB, 0:jn],
                                                  rr[0:1, 0:jn])
                    nc.vector.tensor_mul(osc_t[h][0:VB, 0:jn],
                                         lt[0:VB, 0:jn], rb[0:VB, 0:jn])
                for t in tasks:   # leftovers (shouldn't happen)
                    t()
                pending = de_tasks(j0, jn, osc_t)
            for t in pending:
                t()
        es_c.close()


# ---------------------------------------------------------------- host API

def _prep_inputs(x, freqs_cos, freqs_sin,
                 W_down, W_up, Wq, bq, Wk, bk, Wv, bv, Wo, bo,
                 ft=FT, n_cores=N_CORES, gh=GH, gw=GW, _qdtype=BF16):
    f64 = np.float64
    xT = np.ascontiguousarray(
        np.asarray(x, np.float32).reshape(-1, np.asarray(x).shape[-1]).T)

    pm = _pad_map()
    sw = _swap_perm()
    scale = HD ** -0.5

    def ext_w(W, b, mul=1.0):
        We = np.concatenate(
            [np.asarray(W, f64).T, np.asarray(b, f64)[None, :]],
            axis=0) * mul
        return We  # [EXT, RANK]

    def pad_cols(We):
        out = np.zeros((We.shape[0], PADR), dtype=f64)
        out[:, pm] = We
        return out

    C, S = _rope_tables(np.asarray(freqs_cos, f64),
                        np.asarray(freqs_sin, f64), gh, gw)

    def packed_cs(tab):
        out = np.zeros((128, ft), dtype=f64)
        out[0:HD, :] = tab
        out[64:64 + HD, :] = tab
        return np.ascontiguousarray(out.astype(np.float32))

    # V with per-head blocks [V0..31, ones, V32..47]
    Wve = ext_w(Wv, bv)
    wv_out = np.zeros((EXT, NH * VB), dtype=f64)
    for h in range(NH):
        blk = Wve[:, h * HD:(h + 1) * HD]
        wv_out[:, h * VB:h * VB + OP] = blk[:, 0:OP]
        wv_out[:, h * VB + OP + 1:h * VB + VB] = blk[:, OP:HD]
        wv_out[EXT - 1, h * VB + OP] = 1.0   # ones col via bias row
    # Wo with a zero row at the ones slot of each head block
    WoT = np.asarray(Wo, f64).T              # [RANK, RANK]
    wo_out = np.zeros((NH * VB, RANK), dtype=f64)
    for h in range(NH):
        blk = WoT[h * HD:(h + 1) * HD, :]
        wo_out[h * VB:h * VB + OP, :] = blk[0:OP, :]
        wo_out[h * VB + OP + 1:h * VB + VB, :] = blk[OP:HD, :]

    def b16(a):
        return np.ascontiguousarray(a.astype(_qdtype))

    shared = dict(
        wd=b16(np.asarray(W_down, f64).T),
        wq=b16(pad_cols(ext_w(Wq, bq, scale))),
        wk=b16(pad_cols(ext_w(Wk, bk))),
        wqs=b16(pad_cols(ext_w(np.asarray(Wq)[sw], np.asarray(bq)[sw],
                               scale))),
        wks=b16(pad_cols(ext_w(np.asarray(Wk)[sw], np.asarray(bk)[sw]))),
        wv=b16(wv_out),
        wo=b16(wo_out),
        bo=np.ascontiguousarray(np.asarray(bo, np.float32)[None, :]),
        wu=b16(np.asarray(W_up, f64).T),
        ct=packed_cs(C), st=packed_cs(S),
    )
    in_maps = []
    for c in range(n_cores):
        m = dict(shared)
        m["xt"] = b16(xT[:, c * ft:(c + 1) * ft].astype(f64))
        in_maps.append(m)
    return in_maps


def kernel(x, seq_lens, t_size, h_size, w_size, sequence_cond_compressed_indices,
           freqs_cos, freqs_sin, W_down, W_up, Wq, bq, Wk, bk, Wv, bv, Wo, bo,
           _trace=False):
    from concourse.bass_utils import run_bass_kernel_spmd

    key = "nc_v2"
    if key not in _EXEC_CACHE:
        _EXEC_CACHE[key] = build_nc()
    nc = _EXEC_CACHE[key]

    in_maps = _prep_inputs(x, freqs_cos, freqs_sin, W_down, W_up,
                           Wq, bq, Wk, bk, Wv, bv, Wo, bo)
    kwargs = {}
    if _trace:
        import concourse.bass_utils as bu
        bu.upload_artifacts = lambda tmpdir: tmpdir
        kwargs = dict(trace=True)
    res = run_bass_kernel_spmd(nc, in_maps, core_ids=list(range(N_CORES)), **kwargs)
    y = np.concatenate([np.asarray(res.results[c]["y"], dtype=np.float32)
                        for c in range(N_CORES)], axis=0)
    out = y[None, :, :]
    if _trace:
        return out, res
    return out
